# revision 9
# baseline (speedup 1.0000x reference)
"""COIL sparse-attention scoring kernel for 8 Trainium2 NeuronCores (v2).

Strategy
--------
Shard the doc axis (Bd=128) across the 8 cores (16 docs each); qry tensors are
replicated. Exploit the match sparsity: a query position can only score against
doc tokens with the SAME token id, so the full [4096 x 2048] per-core score
matrix is ~99.6% irrelevant.

Host-side index prep (cheap): prune query rows whose id is absent from the
core's doc slab, sort the survivors by id, and cut them into blocks of 128.
Each block touches ~31 distinct ids, so only ~60 of the core's 2048 doc tokens
can match it. Those tokens are gathered per block (grouped by doc, zero-padded
to a fixed per-doc width P) giving a [128, 16*P] score tile instead of
[128, 2048] -- a ~12x reduction in matmul columns and reduce input.

The exact-match mask folds into the matmul: ids are rank-encoded per block
(dense rank over the block's id set) as two base-B digit one-hots scaled by
ALPHA=32 and appended to the bf16 reps, so

    v[r, c] = S[r, c] + 1024 * match_digits   (match_digits == 2 iff equal id)

and tok = max(v_max, OFF) - OFF with OFF=2048 reproduces the reference
masked-max (pad columns give v = S' + <=1024 < OFF, clamped to 0).

Per group of 4 tiles (one PSUM [128, 2*512] region, 2 tiles per bank):
either a direct DVE reduce_max straight from PSUM f32 + a tiny
tensor_scalar(max OFF, -OFF), or a ScalarE relu(v-OFF)->fp16 followed by a
packed fp16 DVE reduce_max. The per-query sum over rows is a selector matmul
(stationary fp16 0/1 membership matrix); CLS scores and the final 8-way max
run on host (a few thousand elements).
"""

import math
import os
import numpy as np
import ml_dtypes

Bq, Sq, Bd, Sd, D, Dc = 8, 512, 128, 128, 32, 768
NCORES = 8
BD_PER = Bd // NCORES          # 16 docs per core
ALPHA = 32.0
OFF = 2.0 * ALPHA * ALPHA      # 2048: offset of a full 2-digit rank match
GROUP = int(os.environ.get("KERNEL_GROUP", "4"))
# group g is a direct-DVE-reduce group iff g % DIRECT_PERIOD == PHASE
DIRECT_PERIOD = int(os.environ.get("KERNEL_DIRECT_PERIOD", "3"))
DIRECT_PHASE = int(os.environ.get("KERNEL_DIRECT_PHASE", "0"))
WARMUP_MMS = int(os.environ.get("KERNEL_WARMUP_MMS", "4"))
# scalar-path group max: InstPool (may hit the DVE 2x/4x fp16 modes that
# InstTensorReduce lacks) vs InstTensorReduce
USE_POOL = os.environ.get("KERNEL_USE_POOL", "1") == "1"
# walrus semaphore budget: the NEFF epilogue resets every allocated semaphore
# one instruction at a time (~115ns each on the slowest engine), so fewer
# semaphores = shorter fixed tail. 0 = leave walrus default.
MAX_SEMS = int(os.environ.get("KERNEL_MAX_SEMS", "0"))

_CACHE = {}


def _bf16(x):
    return x.astype(ml_dtypes.bfloat16)


def _qry_row_mask(inputs):
    """[Bq, Sq] bool: rows that can contribute (attended, not CLS/SEP)."""
    mask = np.asarray(inputs["qry_attention_mask"], np.int64).copy()
    sep = mask.sum(axis=1) - 1
    mask[np.arange(Bq), sep] = 0
    mask[:, 0] = 0
    return mask.astype(bool)


def _supergroups(nt):
    """Final-sum groups: up to 8 tiles share one selector matmul."""
    return [range(g, min(g + 8, nt)) for g in range(0, nt, 8)]


def _prepare(inputs):
    """Build the per-core packed operands + the compile-time geometry.

    Returns (geom, in_maps) where geom is hashable and fully determines the
    Bass program; in_maps is the per-core dict of dram tensors.
    """
    qry_reps = np.asarray(inputs["qry_reps"], np.float32).reshape(-1, D)
    qry_ids = np.asarray(inputs["qry_input_ids"], np.int64).reshape(-1)
    doc_reps = np.asarray(inputs["doc_reps"], np.float32)
    doc_ids = np.asarray(inputs["doc_input_ids"], np.int64)
    row_ok = _qry_row_mask(inputs).reshape(-1)
    qpos_q = np.repeat(np.arange(Bq), Sq)

    rows_per_core = []
    for core in range(NCORES):
        sl = slice(core * BD_PER, (core + 1) * BD_PER)
        vocab = np.zeros(1000, dtype=bool)
        vocab[doc_ids[sl].reshape(-1)] = True
        rows = np.nonzero(row_ok & vocab[qry_ids])[0]
        rows = rows[np.argsort(qry_ids[rows], kind="stable")]
        rows_per_core.append(rows)
    nt = max((len(r) + 127) // 128 for r in rows_per_core)

    # per (core, tile): id set + per-doc matching token count
    idsets = [[None] * nt for _ in range(NCORES)]
    maxdist = 1
    P_ct = np.zeros((NCORES, nt), dtype=np.int64)
    for core in range(NCORES):
        dids2 = doc_ids[core * BD_PER : (core + 1) * BD_PER]
        rows = rows_per_core[core]
        for t in range(nt):
            rr = rows[t * 128 : (t + 1) * 128]
            if len(rr) == 0:
                idsets[core][t] = np.zeros(0, np.int64)
                continue
            idset = np.unique(qry_ids[rr])
            idsets[core][t] = idset
            maxdist = max(maxdist, len(idset))
            P_ct[core, t] = np.isin(dids2, idset).sum(axis=1).max()
    base = max(7, math.ceil(math.sqrt(maxdist)))
    ndig = 2 * base
    kext = D + ndig

    # group geometry (uniform across cores)
    groups = []
    for t0 in range(0, nt, GROUP):
        ntiles = min(GROUP, nt - t0)
        P = max(1, int(P_ct[:, t0 : t0 + ntiles].max()))
        ds = 1
        while (BD_PER // ds) * P > 512:
            ds *= 2
        groups.append((ntiles, P, ds))
    geom = (kext, base, nt, tuple(groups))

    # column packing
    totcol = sum(ntiles * ds * (BD_PER // ds) * P for ntiles, P, ds in groups)

    in_maps = []
    for core in range(NCORES):
        rows = rows_per_core[core]
        dreps = doc_reps[core * BD_PER : (core + 1) * BD_PER].reshape(-1, D)
        dids = doc_ids[core * BD_PER : (core + 1) * BD_PER].reshape(-1)
        dreps_bf = _bf16(dreps).astype(np.float32)
        qreps_bf = _bf16(qry_reps).astype(np.float32)

        qryT = np.zeros((kext, nt * 128), dtype=np.float32)
        docT = np.zeros((kext, totcol), dtype=np.float32)
        selT = np.zeros((128, 8 * nt), dtype=np.float32)
        col = 0
        for g, (ntiles, P, ds) in enumerate(groups):
            dps = BD_PER // ds
            for tl in range(ntiles):
                t = g * GROUP + tl
                rr = rows[t * 128 : (t + 1) * 128]
                nr = len(rr)
                idset = idsets[core][t]
                if nr:
                    rank_lookup = np.full(1000, -1, np.int64)
                    rank_lookup[idset] = np.arange(len(idset))
                    rk = rank_lookup[qry_ids[rr]]
                    c0 = t * 128
                    qryT[:D, c0 : c0 + nr] = qreps_bf[rr].T
                    qryT[D + rk % base, c0 + np.arange(nr)] = ALPHA
                    qryT[D + base + rk // base, c0 + np.arange(nr)] = ALPHA
                    selT[np.arange(nr), t * 8 + qpos_q[rr]] = 1.0
                    tokmask = np.isin(
                        dids.reshape(BD_PER, Sd), idset
                    )
                else:
                    tokmask = np.zeros((BD_PER, Sd), dtype=bool)
                # doc columns: sub-major (docs split ds ways), doc-major, pad P
                for h in range(ds):
                    for dd in range(dps):
                        d = h * dps + dd
                        js = np.nonzero(tokmask[d])[0]
                        cc = col + h * dps * P + dd * P
                        if len(js):
                            docT[:D, cc : cc + len(js)] = dreps_bf[
                                d * Sd + js
                            ].T
                            rk = rank_lookup[dids[d * Sd + js]]
                            docT[D + rk % base, cc + np.arange(len(js))] = ALPHA
                            docT[
                                D + base + rk // base, cc + np.arange(len(js))
                            ] = ALPHA
                    # half h occupies cols [col + h*dps*P, col + (h+1)*dps*P)
                col += ds * dps * P
        in_maps.append(
            {
                "qryT": _bf16(qryT),
                "docT": _bf16(docT),
                "selT": selT.astype(np.float16),
            }
        )
    return geom, in_maps


_LDW_PATCHED = False


def _patch_ldw_opt():
    """Append extra walrus args (opt-in via env)."""
    global _LDW_PATCHED
    extra = []
    if os.environ.get("KERNEL_LDW_OPT"):
        extra.append("--enable-ldw-opt=true")
    if MAX_SEMS:
        extra.append(f"--max-sem-num={MAX_SEMS}")
    if _LDW_PATCHED or not extra:
        return
    import concourse.bass_utils as bu

    orig = bu.get_walrus_args

    def patched(*a, **k):
        return orig(*a, **k) + extra

    bu.get_walrus_args = patched
    _LDW_PATCHED = True


def _split_multi_waits(nc, mybir):
    """This container's walrus accepts only ONE sync-wait per instruction.
    Hoist extra waits into standalone EventSemaphore instructions on the same
    engine right before the offender (sequencer blocks on each in order)."""
    n = 0
    for func in nc.m.functions:
        for bb in func.blocks:
            out = []
            for inst in bb.instructions:
                si = inst.sync_info
                if si is not None and len(si.on_wait) > 1:
                    waits = list(si.on_wait)
                    for w in waits[:-1]:
                        n += 1
                        out.append(
                            mybir.InstEventSemaphore(
                                name=f"W-{inst.name}-{n}",
                                engine=inst.engine,
                                ins=[],
                                outs=[],
                                debug=inst.debug,
                                sync_info=mybir.SyncInfo(
                                    on_wait=[w], on_update=[]
                                ),
                            )
                        )
                    inst.sync_info = mybir.SyncInfo(
                        on_wait=[waits[-1]], on_update=list(si.on_update)
                    )
                out.append(inst)
            bb.instructions = out
    return n


def _build_nc(geom):
    import concourse.bass as bass
    import concourse.mybir as mybir
    import concourse.tile as tile

    kext, base, nt, groups = geom
    bf16, f16, f32 = mybir.dt.bfloat16, mybir.dt.float16, mybir.dt.float32
    nc = bass.Bass("TRN2", target_bir_lowering=False, debug=False)

    # per-group packing info
    ginfo = []  # (t0, ntiles, P, ds, dps, Ws, per_bank, nb, colofs, gcols)
    col = 0
    nb_max = 1
    for g, (ntiles, P, ds) in enumerate(groups):
        dps = BD_PER // ds
        Ws = dps * P
        nsubs = ntiles * ds
        per_bank = max(1, 512 // Ws) if ds == 1 else 1
        nb = (nsubs + per_bank - 1) // per_bank
        nb_max = max(nb_max, nb)
        gcols = nsubs * Ws
        ginfo.append((g * GROUP, ntiles, P, ds, dps, Ws, per_bank, nb, col, gcols))
        col += gcols
    totcol = col

    qryT = nc.dram_tensor("qryT", [kext, nt * 128], bf16, kind="ExternalInput").ap()
    docT = nc.dram_tensor("docT", [kext, totcol], bf16, kind="ExternalInput").ap()
    selT = nc.dram_tensor("selT", [128, 8 * nt], f16, kind="ExternalInput").ap()
    out = nc.dram_tensor("out", [64, 16 * nt], f16, kind="ExternalOutput").ap()

    n_groups = len(ginfo)
    is_direct = [
        DIRECT_PERIOD > 0 and g % DIRECT_PERIOD == DIRECT_PHASE % DIRECT_PERIOD
        for g in range(n_groups)
    ]

    with tile.TileContext(nc) as tc:
        with (
            tc.tile_pool(name="inp", bufs=1) as inp,
            tc.tile_pool(name="psum", bufs=3, space="PSUM") as psum,
            tc.tile_pool(name="fpsum", bufs=2, space="PSUM") as fpsum,
            tc.tile_pool(name="stage", bufs=2) as stp,
            tc.tile_pool(name="accp", bufs=1) as accp,
        ):
            # input SBUF + DMA: group-0 slices first (small, unblock tile 0),
            # then the remainder as one large transfer per tensor.
            qry_sb = inp.tile([kext, nt * 128], bf16)
            doc_sb = inp.tile([kext, totcol], bf16)
            sel_sb = inp.tile([128, 8 * nt], f16)
            g0cols = ginfo[0][9]
            g0q = ginfo[0][1] * 128
            nc.sync.dma_start(doc_sb[:, 0:g0cols], docT[:, 0:g0cols])
            nc.gpsimd.dma_start(qry_sb[:, 0:g0q], qryT[:, 0:g0q])
            nc.sync.dma_start(doc_sb[:, g0cols:totcol], docT[:, g0cols:totcol])
            nc.gpsimd.dma_start(
                qry_sb[:, g0q : nt * 128], qryT[:, g0q : nt * 128]
            )
            nc.gpsimd.dma_start(sel_sb[:], selT[:])

            negoff = accp.tile([128, 1], f32)
            nc.vector.memset(negoff[:], -OFF)
            # tiny dummy activation: pulls the Relu ACT_TABLE_LOAD into the
            # DMA head instead of stalling the first real group
            atl = accp.tile([128, 1], f16)
            nc.scalar.activation(
                atl[:], negoff[:], mybir.ActivationFunctionType.Relu,
                bias=negoff[:],
            )

            # PE warm-up during the DMA head (HAM clock ramp)
            if WARMUP_MMS:
                scratch = inp.tile([kext, 512], bf16)
                nc.vector.memset(scratch[:], 0.0)
                wps = psum.tile([128, 512], f32, tag="score")
                for _ in range(WARMUP_MMS):
                    nc.tensor.matmul(
                        wps[:], scratch[:, 0:128], scratch[:],
                        start=True, stop=True,
                    )

            accum = accp.tile([128, 16 * nt], f16)
            draw = accp.tile([128, 16 * nt], f32)

            for gi, (t0, ntiles, P, ds, dps, Ws, per_bank, nb, colofs, gcols) in (
                enumerate(ginfo)
            ):
                nsubs = ntiles * ds
                ps = psum.tile([128, nb * 512], f32, tag="score")
                # matmuls: sub j -> bank j//per_bank, slot (j%per_bank)*Ws
                for j in range(nsubs):
                    t = t0 + j // ds
                    slot = (j // per_bank) * 512 + (j % per_bank) * Ws
                    sub = colofs + j * Ws
                    nc.tensor.matmul(
                        ps[:, slot : slot + Ws],
                        qry_sb[:, t * 128 : (t + 1) * 128],
                        doc_sb[:, sub : sub + Ws],
                        start=True,
                        stop=True,
                    )

                # PSUM view chunks: (flat_view [p,(banks,)subs,Ws],
                #                    grouped_view [...,d,t], n_subs_in_chunk)
                def psum_chunks():
                    chunks = []
                    if ds == 1:
                        nfull = nsubs // per_bank
                        rem = nsubs % per_bank
                        if nfull:
                            flat = ps[:, 0 : nfull * 512].rearrange(
                                "p (nb c) -> p nb c", c=512
                            )[:, :, 0 : per_bank * Ws].rearrange(
                                "p nb (s c) -> p nb s c", c=Ws
                            )
                            grp = ps[:, 0 : nfull * 512].rearrange(
                                "p (nb c) -> p nb c", c=512
                            )[:, :, 0 : per_bank * Ws].rearrange(
                                "p nb (s d t) -> p nb s d t", d=dps, t=P
                            )
                            chunks.append((flat, grp, nfull * per_bank))
                        if rem:
                            lo = nfull * 512
                            flat = ps[:, lo : lo + rem * Ws].rearrange(
                                "p (s c) -> p s c", c=Ws
                            )
                            grp = ps[:, lo : lo + rem * Ws].rearrange(
                                "p (s d t) -> p s d t", d=dps, t=P
                            )
                            chunks.append((flat, grp, rem))
                    else:
                        # one sub per bank; banks factor as (tile, half)
                        flat = ps[:, 0 : nsubs * 512].rearrange(
                            "p (nt h c) -> p nt h c", h=ds, c=512
                        )[:, :, :, 0:Ws]
                        grp = ps[:, 0 : nsubs * 512].rearrange(
                            "p (nt h c) -> p nt h c", h=ds, c=512
                        )[:, :, :, 0:Ws].rearrange(
                            "p nt h (d t) -> p nt h d t", t=P
                        )
                        chunks.append((flat, grp, nsubs))
                    return chunks

                c0 = t0 * 16
                if is_direct[gi]:
                    dcol = c0
                    for flat, grp, nsub_c in psum_chunks():
                        ncols = nsub_c * dps
                        od = draw[:, dcol : dcol + ncols]
                        if len(grp.shape) == 5:
                            od = od.rearrange(
                                "p (a s d) -> p a s d", d=dps, s=grp.shape[2]
                            )
                        else:
                            od = od.rearrange("p (s d) -> p s d", d=dps)
                        nc.vector.reduce_max(od, grp, axis=mybir.AxisListType.X)
                        dcol += ncols
                    nc.vector.tensor_scalar(
                        accum[:, c0 : c0 + 16 * ntiles],
                        draw[:, c0 : c0 + 16 * ntiles],
                        OFF,
                        -OFF,
                        mybir.AluOpType.max,
                        mybir.AluOpType.add,
                    )
                else:
                    st = stp.tile([128, nsubs * Ws], f16, tag="stage")
                    scol = 0
                    for flat, grp, nsub_c in psum_chunks():
                        w = nsub_c * Ws
                        so = st[:, scol : scol + w]
                        if len(flat.shape) == 4:
                            so = so.rearrange(
                                "p (nb s c) -> p nb s c",
                                nb=flat.shape[1], c=Ws,
                            )
                        else:
                            so = so.rearrange("p (s c) -> p s c", c=Ws)
                        nc.scalar.activation(
                            so, flat,
                            mybir.ActivationFunctionType.Relu,
                            bias=negoff[:],
                        )
                        scol += w
                    oacc = accum[:, c0 : c0 + 16 * ntiles].rearrange(
                        "p (a d) -> p a d", d=dps
                    )
                    sin = st[:].rearrange("p (a d t) -> p a d t", d=dps, t=P)
                    if USE_POOL:
                        nc.vector.pool_max(oacc, sin)
                    else:
                        nc.vector.reduce_max(
                            oacc, sin, axis=mybir.AxisListType.X
                        )

            # per-q partition sums: selector matmul per supergroup of 8 tiles
            osb = accp.tile([64, 16 * nt], f16)
            for g, grp in enumerate(_supergroups(nt)):
                qts = list(grp)
                gn = len(qts)
                c0 = qts[0] * 16
                fin = fpsum.tile([8 * gn, 16 * gn], f32, tag="fin")
                nc.tensor.matmul(
                    fin[:],
                    sel_sb[:, qts[0] * 8 : (qts[-1] + 1) * 8],
                    accum[:, c0 : c0 + 16 * gn],
                    start=True,
                    stop=True,
                )
                if g % 2 == 0:
                    nc.vector.tensor_copy(osb[0 : 8 * gn, c0 : c0 + 16 * gn], fin[:])
                else:
                    nc.scalar.copy(osb[0 : 8 * gn, c0 : c0 + 16 * gn], fin[:])
                nc.sync.dma_start(
                    out[0 : 8 * gn, c0 : c0 + 16 * gn],
                    osb[0 : 8 * gn, c0 : c0 + 16 * gn],
                )
    _split_multi_waits(nc, mybir)
    return nc


def _get_nc(geom):
    _patch_ldw_opt()
    key = (geom, GROUP, DIRECT_PERIOD, DIRECT_PHASE, WARMUP_MMS)
    if key not in _CACHE:
        _CACHE[key] = _build_nc(geom)
    return _CACHE[key]


def _assemble(inputs, results, nt):
    toks = np.zeros((Bq, Bd), dtype=np.float32)
    for core in range(NCORES):
        osb = np.asarray(results[core]["out"], np.float32)  # [64, 16*nt]
        part = np.zeros((Bq, BD_PER), dtype=np.float32)
        for grp in _supergroups(nt):
            for tl, t in enumerate(grp):
                part += osb[8 * tl : 8 * tl + 8, t * 16 : (t + 1) * 16]
        toks[:, core * BD_PER : (core + 1) * BD_PER] = part
    cls = np.asarray(inputs["qry_cls"], np.float32) @ np.asarray(
        inputs["doc_cls"], np.float32
    ).T
    scores = toks + cls
    return scores.max(axis=0).reshape(-1).astype(np.float32)


def _ensure_ntff_hook():
    """This container's antenv lacks axon_hooks; synthesize the module and
    register the ctypes-based NTFF profile hook so trace=True works."""
    import sys
    import types

    if "antenv.axon_hooks" in sys.modules:
        return
    mod = types.ModuleType("antenv.axon_hooks")
    state = {"hook": None}
    mod.set_axon_ntff_profile_hook = lambda h: state.__setitem__("hook", h)
    mod.get_axon_ntff_profile_hook = lambda: state["hook"]
    sys.modules["antenv.axon_hooks"] = mod
    try:
        import antenv

        antenv.axon_hooks = mod
    except ImportError:
        pass
    try:
        from trn_agent_boot.trn_boot import _ntff_profile_via_ctypes

        mod.set_axon_ntff_profile_hook(
            _ntff_profile_via_ctypes("/opt/axon/libaxon_pjrt.so")
        )
    except Exception:
        pass


def run(inputs, trace=False, **kwargs):
    """Run on the 8 NeuronCores; returns (output, BassKernelResults)."""
    from concourse.bass_utils import run_bass_kernel_spmd

    if trace:
        _ensure_ntff_hook()
    geom, in_maps = _prepare(inputs)
    nc = _get_nc(geom)
    res = run_bass_kernel_spmd(
        nc, in_maps, core_ids=list(range(NCORES)), trace=trace, **kwargs
    )
    return _assemble(inputs, res.results, geom[2]), res


def kernel(**inputs) -> np.ndarray:
    out, _ = run(inputs)
    return out


# revision 19
# speedup vs baseline: 1.0923x; 1.0923x over previous
"""COIL sparse-attention scoring kernel for 8 Trainium2 NeuronCores (v2).

Strategy
--------
Shard the doc axis (Bd=128) across the 8 cores (16 docs each); qry tensors are
replicated. Exploit the match sparsity: a query position can only score against
doc tokens with the SAME token id, so the full [4096 x 2048] per-core score
matrix is ~99.6% irrelevant.

Host-side index prep (cheap): prune query rows whose id is absent from the
core's doc slab, sort the survivors by id, and cut them into blocks of 128.
Each block touches ~31 distinct ids, so only ~60 of the core's 2048 doc tokens
can match it. Those tokens are gathered per block (grouped by doc, zero-padded
to a fixed per-doc width P) giving a [128, 16*P] score tile instead of
[128, 2048] -- a ~12x reduction in matmul columns and reduce input.

The exact-match mask folds into the matmul: ids are rank-encoded per block
(dense rank over the block's id set) as two base-B digit one-hots scaled by
ALPHA=32 and appended to the bf16 reps, so

    v[r, c] = S[r, c] + 1024 * match_digits   (match_digits == 2 iff equal id)

and tok = max(v_max, OFF) - OFF with OFF=2048 reproduces the reference
masked-max (pad columns give v = S' + <=1024 < OFF, clamped to 0).

Per group of 4 tiles (one PSUM [128, 2*512] region, 2 tiles per bank):
either a direct DVE reduce_max straight from PSUM f32 + a tiny
tensor_scalar(max OFF, -OFF), or a ScalarE relu(v-OFF)->fp16 followed by a
packed fp16 DVE reduce_max. The per-query sum over rows is a selector matmul
(stationary fp16 0/1 membership matrix); CLS scores and the final 8-way max
run on host (a few thousand elements).
"""

import math
import os
import numpy as np
import ml_dtypes

Bq, Sq, Bd, Sd, D, Dc = 8, 512, 128, 128, 32, 768
NCORES = 8
BD_PER = Bd // NCORES          # 16 docs per core
ALPHA = 32.0
OFF = 2.0 * ALPHA * ALPHA      # 2048: offset of a full 2-digit rank match
GROUP = int(os.environ.get("KERNEL_GROUP", "4"))
# group g is a direct-DVE-reduce group iff g % DIRECT_PERIOD == PHASE
DIRECT_PERIOD = int(os.environ.get("KERNEL_DIRECT_PERIOD", "3"))
DIRECT_PHASE = int(os.environ.get("KERNEL_DIRECT_PHASE", "0"))
WARMUP_MMS = int(os.environ.get("KERNEL_WARMUP_MMS", "4"))
# scalar-path group max: InstPool is rejected by this walrus build on DVE;
# keep opt-in for experiments
USE_POOL = os.environ.get("KERNEL_USE_POOL", "0") == "1"
# docs within a group are sorted by match count and padded per class of
# BD_PER/DOC_CLASSES docs (instead of all 16 to the global max)
DOC_CLASSES = int(os.environ.get("KERNEL_DOC_CLASSES", "4"))
# walrus semaphore budget: the NEFF epilogue resets every allocated semaphore
# one instruction at a time (~115ns each on the slowest engine), so fewer
# semaphores = shorter fixed tail. 0 = leave walrus default.
MAX_SEMS = int(os.environ.get("KERNEL_MAX_SEMS", "0"))

_CACHE = {}


def _bf16(x):
    return x.astype(ml_dtypes.bfloat16)


def _qry_row_mask(inputs):
    """[Bq, Sq] bool: rows that can contribute (attended, not CLS/SEP)."""
    mask = np.asarray(inputs["qry_attention_mask"], np.int64).copy()
    sep = mask.sum(axis=1) - 1
    mask[np.arange(Bq), sep] = 0
    mask[:, 0] = 0
    return mask.astype(bool)


def _supergroups(nt):
    """Final-sum groups: up to 8 tiles share one selector matmul."""
    return [range(g, min(g + 8, nt)) for g in range(0, nt, 8)]


def _prepare(inputs):
    """Build the per-core packed operands + the compile-time geometry.

    Returns (geom, in_maps, perms): geom is hashable and fully determines the
    Bass program; in_maps is the per-core dict of dram tensors; perms[core][g]
    is the doc permutation (sorted by match count) used for group g's columns.
    """
    qry_reps = np.asarray(inputs["qry_reps"], np.float32).reshape(-1, D)
    qry_ids = np.asarray(inputs["qry_input_ids"], np.int64).reshape(-1)
    doc_reps = np.asarray(inputs["doc_reps"], np.float32)
    doc_ids = np.asarray(inputs["doc_input_ids"], np.int64)
    row_ok = _qry_row_mask(inputs).reshape(-1)
    qpos_q = np.repeat(np.arange(Bq), Sq)

    rows_per_core = []
    for core in range(NCORES):
        sl = slice(core * BD_PER, (core + 1) * BD_PER)
        vocab = np.zeros(1000, dtype=bool)
        vocab[doc_ids[sl].reshape(-1)] = True
        rows = np.nonzero(row_ok & vocab[qry_ids])[0]
        rows = rows[np.argsort(qry_ids[rows], kind="stable")]
        rows_per_core.append(rows)
    nt = max((len(r) + 127) // 128 for r in rows_per_core)
    n_groups = (nt + GROUP - 1) // GROUP

    # per (core, tile): id set; per (core, group, doc): match count
    idsets = [[None] * nt for _ in range(NCORES)]
    maxdist = 1
    cnt_cgd = np.zeros((NCORES, n_groups, BD_PER), dtype=np.int64)
    for core in range(NCORES):
        dids2 = doc_ids[core * BD_PER : (core + 1) * BD_PER]
        rows = rows_per_core[core]
        for t in range(nt):
            rr = rows[t * 128 : (t + 1) * 128]
            if len(rr) == 0:
                idsets[core][t] = np.zeros(0, np.int64)
                continue
            idset = np.unique(qry_ids[rr])
            idsets[core][t] = idset
            maxdist = max(maxdist, len(idset))
            cnt_cgd[core, t // GROUP] = np.maximum(
                cnt_cgd[core, t // GROUP], np.isin(dids2, idset).sum(axis=1)
            )
    base = max(7, math.ceil(math.sqrt(maxdist)))
    ndig = 2 * base
    kext = D + ndig

    # doc permutation (count-desc) per (core, group); class widths uniform
    # across cores per (group, class)
    perms = [
        [np.argsort(-cnt_cgd[core, g], kind="stable") for g in range(n_groups)]
        for core in range(NCORES)
    ]
    groups = []
    for g in range(n_groups):
        ntiles = min(GROUP, nt - g * GROUP)
        scnt = np.sort(cnt_cgd[:, g], axis=1)[:, ::-1]  # [cores, BD_PER] desc
        C = DOC_CLASSES
        w = BD_PER // C
        Pks = tuple(int(scnt[:, k * w].max()) for k in range(C))
        if C > 1 and Pks[0] == 0:
            Pks = (1,) + Pks[1:]  # keep at least one nonempty class
        ds = 1
        if sum(w * p for p in Pks) > 512:
            # fall back: single class, split docs across banks, no perm
            P = max(1, int(scnt[:, 0].max()))
            ds = 1
            while (BD_PER // ds) * P > 512:
                ds *= 2
            Pks = (P,)
            for core in range(NCORES):
                perms[core][g] = np.arange(BD_PER)
        groups.append((ntiles, Pks, ds))
    geom = (kext, base, nt, tuple(groups))

    def sub_width(Pks, ds):
        if ds == 1:
            w = BD_PER // len(Pks) if len(Pks) > 1 else BD_PER
            return sum(w * p for p in Pks) if len(Pks) > 1 else BD_PER * Pks[0]
        return (BD_PER // ds) * Pks[0]

    totcol = sum(
        ntiles * ds * sub_width(Pks, ds) for ntiles, Pks, ds in groups
    )

    in_maps = []
    for core in range(NCORES):
        rows = rows_per_core[core]
        dreps = doc_reps[core * BD_PER : (core + 1) * BD_PER].reshape(-1, D)
        dids = doc_ids[core * BD_PER : (core + 1) * BD_PER].reshape(-1)
        dreps_bf = _bf16(dreps).astype(np.float32)
        qreps_bf = _bf16(qry_reps).astype(np.float32)

        qryT = np.zeros((kext, nt * 128), dtype=np.float32)
        docT = np.zeros((kext, totcol), dtype=np.float32)
        selT = np.zeros((128, 8 * nt), dtype=np.float32)
        col = 0
        for g, (ntiles, Pks, ds) in enumerate(groups):
            C = len(Pks)
            w = BD_PER // C if ds == 1 else BD_PER // ds
            perm = perms[core][g]
            for tl in range(ntiles):
                t = g * GROUP + tl
                rr = rows[t * 128 : (t + 1) * 128]
                nr = len(rr)
                idset = idsets[core][t]
                rank_lookup = np.full(1000, -1, np.int64)
                if nr:
                    rank_lookup[idset] = np.arange(len(idset))
                    rk = rank_lookup[qry_ids[rr]]
                    c0 = t * 128
                    qryT[:D, c0 : c0 + nr] = qreps_bf[rr].T
                    qryT[D + rk % base, c0 + np.arange(nr)] = ALPHA
                    qryT[D + base + rk // base, c0 + np.arange(nr)] = ALPHA
                    selT[np.arange(nr), t * 8 + qpos_q[rr]] = 1.0
                    tokmask = np.isin(dids.reshape(BD_PER, Sd), idset)
                else:
                    tokmask = np.zeros((BD_PER, Sd), dtype=bool)

                def put_doc(d, cc, pmax):
                    js = np.nonzero(tokmask[d])[0]
                    assert len(js) <= pmax
                    if len(js):
                        docT[:D, cc : cc + len(js)] = dreps_bf[d * Sd + js].T
                        rk2 = rank_lookup[dids[d * Sd + js]]
                        docT[D + rk2 % base, cc + np.arange(len(js))] = ALPHA
                        docT[
                            D + base + rk2 // base, cc + np.arange(len(js))
                        ] = ALPHA

                if ds == 1:
                    cc = col
                    for k in range(C):
                        for slot in range(w):
                            put_doc(perm[k * w + slot], cc, Pks[k])
                            cc += Pks[k]
                    col = cc
                else:
                    P = Pks[0]
                    for h in range(ds):
                        for dd in range(w):
                            put_doc(h * w + dd, col + (h * w + dd) * P, P)
                    col += ds * w * P
        in_maps.append(
            {
                "qryT": _bf16(qryT),
                "docT": _bf16(docT),
                "selT": selT.astype(np.float16),
            }
        )
    return geom, in_maps, perms


_LDW_PATCHED = False


def _patch_ldw_opt():
    """Append extra walrus args (opt-in via env)."""
    global _LDW_PATCHED
    extra = []
    if os.environ.get("KERNEL_LDW_OPT"):
        extra.append("--enable-ldw-opt=true")
    if MAX_SEMS:
        extra.append(f"--max-sem-num={MAX_SEMS}")
    if _LDW_PATCHED or not extra:
        return
    import concourse.bass_utils as bu

    orig = bu.get_walrus_args

    def patched(*a, **k):
        return orig(*a, **k) + extra

    bu.get_walrus_args = patched
    _LDW_PATCHED = True


def _split_multi_waits(nc, mybir):
    """This container's walrus accepts only ONE sync-wait per instruction.
    Hoist extra waits into standalone EventSemaphore instructions on the same
    engine right before the offender (sequencer blocks on each in order)."""
    n = 0
    for func in nc.m.functions:
        for bb in func.blocks:
            out = []
            for inst in bb.instructions:
                si = inst.sync_info
                if si is not None and len(si.on_wait) > 1:
                    waits = list(si.on_wait)
                    for w in waits[:-1]:
                        n += 1
                        out.append(
                            mybir.InstEventSemaphore(
                                name=f"W-{inst.name}-{n}",
                                engine=inst.engine,
                                ins=[],
                                outs=[],
                                debug=inst.debug,
                                sync_info=mybir.SyncInfo(
                                    on_wait=[w], on_update=[]
                                ),
                            )
                        )
                    inst.sync_info = mybir.SyncInfo(
                        on_wait=[waits[-1]], on_update=list(si.on_update)
                    )
                out.append(inst)
            bb.instructions = out
    return n


def _build_nc(geom):
    import concourse.bass as bass
    import concourse.mybir as mybir
    import concourse.tile as tile

    kext, base, nt, groups = geom
    bf16, f16, f32 = mybir.dt.bfloat16, mybir.dt.float16, mybir.dt.float32
    nc = bass.Bass("TRN2", target_bir_lowering=False, debug=False)

    # per-group packing info
    # (t0, ntiles, Pks, ds, w, Ws, per_bank, nb, colofs, gcols)
    ginfo = []
    col = 0
    for g, (ntiles, Pks, ds) in enumerate(groups):
        C = len(Pks)
        w = (BD_PER // C) if ds == 1 else (BD_PER // ds)
        Ws = sum(w * p for p in Pks)
        nsubs = ntiles * ds
        per_bank = max(1, 512 // Ws) if ds == 1 else 1
        nb = (nsubs + per_bank - 1) // per_bank
        gcols = nsubs * Ws
        ginfo.append((g * GROUP, ntiles, Pks, ds, w, Ws, per_bank, nb, col, gcols))
        col += gcols
    totcol = col

    qryT = nc.dram_tensor("qryT", [kext, nt * 128], bf16, kind="ExternalInput").ap()
    docT = nc.dram_tensor("docT", [kext, totcol], bf16, kind="ExternalInput").ap()
    selT = nc.dram_tensor("selT", [128, 8 * nt], f16, kind="ExternalInput").ap()
    out = nc.dram_tensor("out", [64, 16 * nt], f16, kind="ExternalOutput").ap()

    n_groups = len(ginfo)
    is_direct = [
        DIRECT_PERIOD > 0 and g % DIRECT_PERIOD == DIRECT_PHASE % DIRECT_PERIOD
        for g in range(n_groups)
    ]

    with tile.TileContext(nc) as tc:
        with (
            tc.tile_pool(name="inp", bufs=1) as inp,
            tc.tile_pool(name="psum", bufs=3, space="PSUM") as psum,
            tc.tile_pool(name="fpsum", bufs=2, space="PSUM") as fpsum,
            tc.tile_pool(name="stage", bufs=2) as stp,
            tc.tile_pool(name="accp", bufs=1) as accp,
        ):
            # input SBUF + DMA. Effective DMA bandwidth is ~23 GB/s per DMA
            # engine and each dma_start engages only 2 engines, so spread
            # concurrent transfers across all five issue queues. Group-0
            # slices go first (small, unblock tile 0).
            qry_sb = inp.tile([kext, nt * 128], bf16)
            doc_sb = inp.tile([kext, totcol], bf16)
            sel_sb = inp.tile([128, 8 * nt], f16)
            gb = [gi[8] for gi in ginfo] + [totcol]  # group col offsets
            ng = len(ginfo)
            dA, dB = min(3, ng), min(5, ng)   # doc chunk group boundaries
            qA, qB = min(4 * GROUP, nt), min(6 * GROUP, nt)
            g0q = ginfo[0][1] * 128
            # only SP/Activation (HWDGE) + gpsimd (SWDGE) can issue DMAs;
            # interleave so each queue moves ~1/3 of the bytes and group-0
            # data lands first
            nc.sync.dma_start(doc_sb[:, 0 : gb[1]], docT[:, 0 : gb[1]])
            nc.gpsimd.dma_start(qry_sb[:, 0:g0q], qryT[:, 0:g0q])
            nc.scalar.dma_start(doc_sb[:, gb[1] : gb[dA]], docT[:, gb[1] : gb[dA]])
            nc.gpsimd.dma_start(
                qry_sb[:, g0q : qA * 128], qryT[:, g0q : qA * 128]
            )
            nc.sync.dma_start(doc_sb[:, gb[dA] : gb[dB]], docT[:, gb[dA] : gb[dB]])
            nc.scalar.dma_start(
                qry_sb[:, qA * 128 : qB * 128], qryT[:, qA * 128 : qB * 128]
            )
            if dB < ng:
                nc.gpsimd.dma_start(
                    doc_sb[:, gb[dB] : totcol], docT[:, gb[dB] : totcol]
                )
            if qB < nt:
                nc.sync.dma_start(
                    qry_sb[:, qB * 128 : nt * 128], qryT[:, qB * 128 : nt * 128]
                )
            nc.scalar.dma_start(sel_sb[:], selT[:])

            negoff = accp.tile([128, 1], f32)
            nc.vector.memset(negoff[:], -OFF)
            # tiny dummy activation: pulls the Relu ACT_TABLE_LOAD into the
            # DMA head instead of stalling the first real group
            atl = accp.tile([128, 1], f16)
            nc.scalar.activation(
                atl[:], negoff[:], mybir.ActivationFunctionType.Relu,
                bias=negoff[:],
            )

            # PE warm-up during the DMA head (HAM clock ramp)
            if WARMUP_MMS:
                scratch = inp.tile([kext, 512], bf16)
                nc.vector.memset(scratch[:], 0.0)
                wps = psum.tile([128, 512], f32, tag="score")
                for _ in range(WARMUP_MMS):
                    nc.tensor.matmul(
                        wps[:], scratch[:, 0:128], scratch[:],
                        start=True, stop=True,
                    )

            accum = accp.tile([128, 16 * nt], f16)
            draw = accp.tile([128, 16 * nt], f32)
            if any(0 in gi[2] for gi in ginfo):
                # zero-width classes leave accum/draw cols unwritten
                nc.vector.memset(draw[:], 0.0)
                nc.scalar.memzero(accum[:])

            for gi, (t0, ntiles, Pks, ds, w, Ws, per_bank, nb, colofs, gcols) in (
                enumerate(ginfo)
            ):
                C = len(Pks)
                offk = [sum(w * p for p in Pks[:k]) for k in range(C)]
                nsubs = ntiles * ds
                ps = psum.tile([128, nb * 512], f32, tag="score")
                # matmuls: sub j -> bank j//per_bank, slot (j%per_bank)*Ws
                for j in range(nsubs):
                    t = t0 + j // ds
                    slot = (j // per_bank) * 512 + (j % per_bank) * Ws
                    sub = colofs + j * Ws
                    nc.tensor.matmul(
                        ps[:, slot : slot + Ws],
                        qry_sb[:, t * 128 : (t + 1) * 128],
                        doc_sb[:, sub : sub + Ws],
                        start=True,
                        stop=True,
                    )

                # chunks of subs with a regular bank pattern:
                # (bank0, nbanks, subs_per_bank, sub0)
                if ds == 1:
                    nfull = nsubs // per_bank
                    rem = nsubs % per_bank
                    chunks = []
                    if nfull:
                        chunks.append((0, nfull, per_bank, 0))
                    if rem:
                        chunks.append((nfull, 1, rem, nfull * per_bank))
                else:
                    chunks = [(0, nsubs, 1, 0)]

                def flat_view(b0, nbc, sc):
                    """[p, nb, s, Ws] strided view of the chunk's PSUM."""
                    return ps[:, b0 * 512 : (b0 + nbc) * 512].rearrange(
                        "p (nb c) -> p nb c", c=512
                    )[:, :, 0 : sc * Ws].rearrange(
                        "p nb (s c) -> p nb s c", c=Ws
                    )

                c0 = t0 * 16
                subcols = 16 // ds  # accum cols per sub
                if is_direct[gi]:
                    for b0, nbc, sc, s0 in chunks:
                        fv = flat_view(b0, nbc, sc)
                        ob = draw[
                            :, c0 + s0 * subcols : c0 + (s0 + nbc * sc) * subcols
                        ].rearrange("p (nb s c) -> p nb s c", nb=nbc, c=subcols)
                        for k in range(C):
                            if Pks[k] == 0:
                                continue
                            nc.vector.reduce_max(
                                ob[:, :, :, k * w : (k + 1) * w],
                                fv[
                                    :, :, :, offk[k] : offk[k] + w * Pks[k]
                                ].rearrange("p nb s (d t) -> p nb s d t", t=Pks[k]),
                                axis=mybir.AxisListType.X,
                            )
                    nc.vector.tensor_scalar(
                        accum[:, c0 : c0 + 16 * ntiles],
                        draw[:, c0 : c0 + 16 * ntiles],
                        OFF,
                        -OFF,
                        mybir.AluOpType.max,
                        mybir.AluOpType.add,
                    )
                else:
                    st = stp.tile([128, nsubs * Ws], f16, tag="stage")
                    for b0, nbc, sc, s0 in chunks:
                        so = st[:, s0 * Ws : (s0 + nbc * sc) * Ws].rearrange(
                            "p (nb s c) -> p nb s c", nb=nbc, c=Ws
                        )
                        nc.scalar.activation(
                            so, flat_view(b0, nbc, sc),
                            mybir.ActivationFunctionType.Relu,
                            bias=negoff[:],
                        )
                    sv = st[:].rearrange("p (a c) -> p a c", c=Ws)
                    oacc = accum[:, c0 : c0 + 16 * ntiles].rearrange(
                        "p (a c) -> p a c", c=subcols
                    )
                    for k in range(C):
                        if Pks[k] == 0:
                            continue
                        sin = sv[:, :, offk[k] : offk[k] + w * Pks[k]].rearrange(
                            "p a (d t) -> p a d t", t=Pks[k]
                        )
                        ok = oacc[:, :, k * w : (k + 1) * w]
                        if USE_POOL:
                            nc.vector.pool_max(ok, sin)
                        else:
                            nc.vector.reduce_max(
                                ok, sin, axis=mybir.AxisListType.X
                            )

            # per-q partition sums: selector matmul per supergroup of 8 tiles
            osb = accp.tile([64, 16 * nt], f16)
            for g, grp in enumerate(_supergroups(nt)):
                qts = list(grp)
                gn = len(qts)
                c0 = qts[0] * 16
                fin = fpsum.tile([8 * gn, 16 * gn], f32, tag="fin")
                nc.tensor.matmul(
                    fin[:],
                    sel_sb[:, qts[0] * 8 : (qts[-1] + 1) * 8],
                    accum[:, c0 : c0 + 16 * gn],
                    start=True,
                    stop=True,
                )
                if g % 2 == 0:
                    nc.vector.tensor_copy(osb[0 : 8 * gn, c0 : c0 + 16 * gn], fin[:])
                else:
                    nc.scalar.copy(osb[0 : 8 * gn, c0 : c0 + 16 * gn], fin[:])
                nc.sync.dma_start(
                    out[0 : 8 * gn, c0 : c0 + 16 * gn],
                    osb[0 : 8 * gn, c0 : c0 + 16 * gn],
                )
    _split_multi_waits(nc, mybir)
    return nc


def _get_nc(geom):
    _patch_ldw_opt()
    key = (geom, GROUP, DIRECT_PERIOD, DIRECT_PHASE, WARMUP_MMS)
    if key not in _CACHE:
        _CACHE[key] = _build_nc(geom)
    return _CACHE[key]


def _assemble(inputs, results, nt, perms):
    toks = np.zeros((Bq, Bd), dtype=np.float32)
    for core in range(NCORES):
        osb = np.asarray(results[core]["out"], np.float32)  # [64, 16*nt]
        part = np.zeros((Bq, BD_PER), dtype=np.float32)
        for grp in _supergroups(nt):
            for tl, t in enumerate(grp):
                part[:, perms[core][t // GROUP]] += osb[
                    8 * tl : 8 * tl + 8, t * 16 : (t + 1) * 16
                ]
        toks[:, core * BD_PER : (core + 1) * BD_PER] = part
    cls = np.asarray(inputs["qry_cls"], np.float32) @ np.asarray(
        inputs["doc_cls"], np.float32
    ).T
    scores = toks + cls
    return scores.max(axis=0).reshape(-1).astype(np.float32)


def _ensure_ntff_hook():
    """This container's antenv lacks axon_hooks; synthesize the module and
    register the ctypes-based NTFF profile hook so trace=True works."""
    import sys
    import types

    if "antenv.axon_hooks" in sys.modules:
        return
    mod = types.ModuleType("antenv.axon_hooks")
    state = {"hook": None}
    mod.set_axon_ntff_profile_hook = lambda h: state.__setitem__("hook", h)
    mod.get_axon_ntff_profile_hook = lambda: state["hook"]
    sys.modules["antenv.axon_hooks"] = mod
    try:
        import antenv

        antenv.axon_hooks = mod
    except ImportError:
        pass
    try:
        from trn_agent_boot.trn_boot import _ntff_profile_via_ctypes

        mod.set_axon_ntff_profile_hook(
            _ntff_profile_via_ctypes("/opt/axon/libaxon_pjrt.so")
        )
    except Exception:
        pass


def run(inputs, trace=False, **kwargs):
    """Run on the 8 NeuronCores; returns (output, BassKernelResults)."""
    from concourse.bass_utils import run_bass_kernel_spmd

    if trace:
        _ensure_ntff_hook()
    geom, in_maps, perms = _prepare(inputs)
    nc = _get_nc(geom)
    res = run_bass_kernel_spmd(
        nc, in_maps, core_ids=list(range(NCORES)), trace=trace, **kwargs
    )
    return _assemble(inputs, res.results, geom[2], perms), res


def kernel(**inputs) -> np.ndarray:
    out, _ = run(inputs)
    return out


# revision 20
# speedup vs baseline: 1.1802x; 1.0804x over previous
"""COIL sparse-attention scoring kernel for 8 Trainium2 NeuronCores (v2).

Strategy
--------
Shard the doc axis (Bd=128) across the 8 cores (16 docs each); qry tensors are
replicated. Exploit the match sparsity: a query position can only score against
doc tokens with the SAME token id, so the full [4096 x 2048] per-core score
matrix is ~99.6% irrelevant.

Host-side index prep (cheap): prune query rows whose id is absent from the
core's doc slab, sort the survivors by id, and cut them into blocks of 128.
Each block touches ~31 distinct ids, so only ~60 of the core's 2048 doc tokens
can match it. Those tokens are gathered per block (grouped by doc, zero-padded
to a fixed per-doc width P) giving a [128, 16*P] score tile instead of
[128, 2048] -- a ~12x reduction in matmul columns and reduce input.

The exact-match mask folds into the matmul: ids are rank-encoded per block
(dense rank over the block's id set) as two base-B digit one-hots scaled by
ALPHA=32 and appended to the bf16 reps, so

    v[r, c] = S[r, c] + 1024 * match_digits   (match_digits == 2 iff equal id)

and tok = max(v_max, OFF) - OFF with OFF=2048 reproduces the reference
masked-max (pad columns give v = S' + <=1024 < OFF, clamped to 0).

Per group of 4 tiles (one PSUM [128, 2*512] region, 2 tiles per bank):
either a direct DVE reduce_max straight from PSUM f32 + a tiny
tensor_scalar(max OFF, -OFF), or a ScalarE relu(v-OFF)->fp16 followed by a
packed fp16 DVE reduce_max. The per-query sum over rows is a selector matmul
(stationary fp16 0/1 membership matrix); CLS scores and the final 8-way max
run on host (a few thousand elements).
"""

import math
import os
import numpy as np
import ml_dtypes

Bq, Sq, Bd, Sd, D, Dc = 8, 512, 128, 128, 32, 768
NCORES = 8
BD_PER = Bd // NCORES          # 16 docs per core
ALPHA = 32.0
OFF = 2.0 * ALPHA * ALPHA      # 2048: offset of a full 2-digit rank match
GROUP = int(os.environ.get("KERNEL_GROUP", "4"))
# group g is a direct-DVE-reduce group iff g % DIRECT_PERIOD == PHASE
DIRECT_PERIOD = int(os.environ.get("KERNEL_DIRECT_PERIOD", "3"))
DIRECT_PHASE = int(os.environ.get("KERNEL_DIRECT_PHASE", "0"))
WARMUP_MMS = int(os.environ.get("KERNEL_WARMUP_MMS", "4"))
# scalar-path group max: InstPool is rejected by this walrus build on DVE;
# keep opt-in for experiments
USE_POOL = os.environ.get("KERNEL_USE_POOL", "0") == "1"
# docs within a group are sorted by match count and padded per class of
# BD_PER/DOC_CLASSES docs (instead of all 16 to the global max)
DOC_CLASSES = int(os.environ.get("KERNEL_DOC_CLASSES", "4"))
# walrus semaphore budget: the NEFF epilogue resets every allocated semaphore
# one instruction at a time (~115ns each on the slowest engine), so fewer
# semaphores = shorter fixed tail. 0 = leave walrus default.
MAX_SEMS = int(os.environ.get("KERNEL_MAX_SEMS", "0"))

_CACHE = {}


def _bf16(x):
    return x.astype(ml_dtypes.bfloat16)


def _qry_row_mask(inputs):
    """[Bq, Sq] bool: rows that can contribute (attended, not CLS/SEP)."""
    mask = np.asarray(inputs["qry_attention_mask"], np.int64).copy()
    sep = mask.sum(axis=1) - 1
    mask[np.arange(Bq), sep] = 0
    mask[:, 0] = 0
    return mask.astype(bool)


def _supergroups(nt):
    """Final-sum groups: up to 8 tiles share one selector matmul."""
    return [range(g, min(g + 8, nt)) for g in range(0, nt, 8)]


def _prepare(inputs):
    """Build the per-core packed operands + the compile-time geometry.

    Returns (geom, in_maps, perms): geom is hashable and fully determines the
    Bass program; in_maps is the per-core dict of dram tensors; perms[core][g]
    is the doc permutation (sorted by match count) used for group g's columns.
    """
    qry_reps = np.asarray(inputs["qry_reps"], np.float32).reshape(-1, D)
    qry_ids = np.asarray(inputs["qry_input_ids"], np.int64).reshape(-1)
    doc_reps = np.asarray(inputs["doc_reps"], np.float32)
    doc_ids = np.asarray(inputs["doc_input_ids"], np.int64)
    row_ok = _qry_row_mask(inputs).reshape(-1)
    qpos_q = np.repeat(np.arange(Bq), Sq)

    rows_per_core = []
    for core in range(NCORES):
        sl = slice(core * BD_PER, (core + 1) * BD_PER)
        vocab = np.zeros(1000, dtype=bool)
        vocab[doc_ids[sl].reshape(-1)] = True
        rows = np.nonzero(row_ok & vocab[qry_ids])[0]
        rows = rows[np.argsort(qry_ids[rows], kind="stable")]
        rows_per_core.append(rows)
    nt = max((len(r) + 127) // 128 for r in rows_per_core)
    n_groups = (nt + GROUP - 1) // GROUP

    # per (core, tile): id set; per (core, group, doc): match count
    idsets = [[None] * nt for _ in range(NCORES)]
    maxdist = 1
    cnt_cgd = np.zeros((NCORES, n_groups, BD_PER), dtype=np.int64)
    for core in range(NCORES):
        dids2 = doc_ids[core * BD_PER : (core + 1) * BD_PER]
        rows = rows_per_core[core]
        for t in range(nt):
            rr = rows[t * 128 : (t + 1) * 128]
            if len(rr) == 0:
                idsets[core][t] = np.zeros(0, np.int64)
                continue
            idset = np.unique(qry_ids[rr])
            idsets[core][t] = idset
            maxdist = max(maxdist, len(idset))
            cnt_cgd[core, t // GROUP] = np.maximum(
                cnt_cgd[core, t // GROUP], np.isin(dids2, idset).sum(axis=1)
            )
    base = max(7, math.ceil(math.sqrt(maxdist)))
    ndig = 2 * base
    kext = D + ndig

    # doc permutation (count-desc) per (core, group); class widths uniform
    # across cores per (group, class)
    perms = [
        [np.argsort(-cnt_cgd[core, g], kind="stable") for g in range(n_groups)]
        for core in range(NCORES)
    ]
    groups = []
    for g in range(n_groups):
        ntiles = min(GROUP, nt - g * GROUP)
        scnt = np.sort(cnt_cgd[:, g], axis=1)[:, ::-1]  # [cores, BD_PER] desc
        C = DOC_CLASSES
        w = BD_PER // C
        Pks = tuple(int(scnt[:, k * w].max()) for k in range(C))
        if C > 1 and Pks[0] == 0:
            Pks = (1,) + Pks[1:]  # keep at least one nonempty class
        ds = 1
        if sum(w * p for p in Pks) > 512:
            # fall back: single class, split docs across banks, no perm
            P = max(1, int(scnt[:, 0].max()))
            ds = 1
            while (BD_PER // ds) * P > 512:
                ds *= 2
            Pks = (P,)
            for core in range(NCORES):
                perms[core][g] = np.arange(BD_PER)
        groups.append((ntiles, Pks, ds))
    geom = (kext, base, nt, tuple(groups))

    def sub_width(Pks, ds):
        if ds == 1:
            w = BD_PER // len(Pks) if len(Pks) > 1 else BD_PER
            return sum(w * p for p in Pks) if len(Pks) > 1 else BD_PER * Pks[0]
        return (BD_PER // ds) * Pks[0]

    totcol = sum(
        ntiles * ds * sub_width(Pks, ds) for ntiles, Pks, ds in groups
    )

    in_maps = []
    for core in range(NCORES):
        rows = rows_per_core[core]
        dreps = doc_reps[core * BD_PER : (core + 1) * BD_PER].reshape(-1, D)
        dids = doc_ids[core * BD_PER : (core + 1) * BD_PER].reshape(-1)
        dreps_bf = _bf16(dreps).astype(np.float32)
        qreps_bf = _bf16(qry_reps).astype(np.float32)

        qryT = np.zeros((kext, nt * 128), dtype=np.float32)
        docT = np.zeros((kext, totcol), dtype=np.float32)
        selT = np.zeros((128, 8 * nt), dtype=np.float32)
        col = 0
        for g, (ntiles, Pks, ds) in enumerate(groups):
            C = len(Pks)
            w = BD_PER // C if ds == 1 else BD_PER // ds
            perm = perms[core][g]
            for tl in range(ntiles):
                t = g * GROUP + tl
                rr = rows[t * 128 : (t + 1) * 128]
                nr = len(rr)
                idset = idsets[core][t]
                rank_lookup = np.full(1000, -1, np.int64)
                if nr:
                    rank_lookup[idset] = np.arange(len(idset))
                    rk = rank_lookup[qry_ids[rr]]
                    c0 = t * 128
                    qryT[:D, c0 : c0 + nr] = qreps_bf[rr].T
                    qryT[D + rk % base, c0 + np.arange(nr)] = ALPHA
                    qryT[D + base + rk // base, c0 + np.arange(nr)] = ALPHA
                    selT[np.arange(nr), t * 8 + qpos_q[rr]] = 1.0
                    tokmask = np.isin(dids.reshape(BD_PER, Sd), idset)
                else:
                    tokmask = np.zeros((BD_PER, Sd), dtype=bool)

                def put_doc(d, cc, pmax):
                    js = np.nonzero(tokmask[d])[0]
                    assert len(js) <= pmax
                    if len(js):
                        docT[:D, cc : cc + len(js)] = dreps_bf[d * Sd + js].T
                        rk2 = rank_lookup[dids[d * Sd + js]]
                        docT[D + rk2 % base, cc + np.arange(len(js))] = ALPHA
                        docT[
                            D + base + rk2 // base, cc + np.arange(len(js))
                        ] = ALPHA

                if ds == 1:
                    cc = col
                    for k in range(C):
                        for slot in range(w):
                            put_doc(perm[k * w + slot], cc, Pks[k])
                            cc += Pks[k]
                    col = cc
                else:
                    P = Pks[0]
                    for h in range(ds):
                        for dd in range(w):
                            put_doc(h * w + dd, col + (h * w + dd) * P, P)
                    col += ds * w * P
        in_maps.append(
            {
                "qryT": _bf16(qryT),
                "docT": _bf16(docT),
                "selT": selT.astype(np.float16),
            }
        )
    return geom, in_maps, perms


_LDW_PATCHED = False


def _patch_ldw_opt():
    """Append extra walrus args (opt-in via env)."""
    global _LDW_PATCHED
    extra = []
    if os.environ.get("KERNEL_LDW_OPT"):
        extra.append("--enable-ldw-opt=true")
    if MAX_SEMS:
        extra.append(f"--max-sem-num={MAX_SEMS}")
    if _LDW_PATCHED or not extra:
        return
    import concourse.bass_utils as bu

    orig = bu.get_walrus_args

    def patched(*a, **k):
        return orig(*a, **k) + extra

    bu.get_walrus_args = patched
    _LDW_PATCHED = True


def _split_multi_waits(nc, mybir):
    """This container's walrus accepts only ONE sync-wait per instruction.
    Hoist extra waits into standalone EventSemaphore instructions on the same
    engine right before the offender (sequencer blocks on each in order)."""
    n = 0
    for func in nc.m.functions:
        for bb in func.blocks:
            out = []
            for inst in bb.instructions:
                si = inst.sync_info
                if si is not None and len(si.on_wait) > 1:
                    waits = list(si.on_wait)
                    for w in waits[:-1]:
                        n += 1
                        out.append(
                            mybir.InstEventSemaphore(
                                name=f"W-{inst.name}-{n}",
                                engine=inst.engine,
                                ins=[],
                                outs=[],
                                debug=inst.debug,
                                sync_info=mybir.SyncInfo(
                                    on_wait=[w], on_update=[]
                                ),
                            )
                        )
                    inst.sync_info = mybir.SyncInfo(
                        on_wait=[waits[-1]], on_update=list(si.on_update)
                    )
                out.append(inst)
            bb.instructions = out
    return n


def _build_nc(geom):
    import concourse.bass as bass
    import concourse.mybir as mybir
    import concourse.tile as tile

    kext, base, nt, groups = geom
    bf16, f16, f32 = mybir.dt.bfloat16, mybir.dt.float16, mybir.dt.float32
    nc = bass.Bass("TRN2", target_bir_lowering=False, debug=False)

    # per-group packing info
    # (t0, ntiles, Pks, ds, w, Ws, per_bank, nb, colofs, gcols)
    ginfo = []
    col = 0
    for g, (ntiles, Pks, ds) in enumerate(groups):
        C = len(Pks)
        w = (BD_PER // C) if ds == 1 else (BD_PER // ds)
        Ws = sum(w * p for p in Pks)
        nsubs = ntiles * ds
        per_bank = max(1, 512 // Ws) if ds == 1 else 1
        nb = (nsubs + per_bank - 1) // per_bank
        gcols = nsubs * Ws
        ginfo.append((g * GROUP, ntiles, Pks, ds, w, Ws, per_bank, nb, col, gcols))
        col += gcols
    totcol = col

    qryT = nc.dram_tensor("qryT", [kext, nt * 128], bf16, kind="ExternalInput").ap()
    docT = nc.dram_tensor("docT", [kext, totcol], bf16, kind="ExternalInput").ap()
    selT = nc.dram_tensor("selT", [128, 8 * nt], f16, kind="ExternalInput").ap()
    out = nc.dram_tensor("out", [64, 16 * nt], f16, kind="ExternalOutput").ap()

    n_groups = len(ginfo)
    is_direct = [
        DIRECT_PERIOD > 0 and g % DIRECT_PERIOD == DIRECT_PHASE % DIRECT_PERIOD
        for g in range(n_groups)
    ]

    with tile.TileContext(nc) as tc:
        with (
            tc.tile_pool(name="inp", bufs=1) as inp,
            tc.tile_pool(name="psum", bufs=3, space="PSUM") as psum,
            tc.tile_pool(name="fpsum", bufs=2, space="PSUM") as fpsum,
            tc.tile_pool(name="stage", bufs=2) as stp,
            tc.tile_pool(name="accp", bufs=1) as accp,
        ):
            # input SBUF + DMA. Effective DMA bandwidth is ~23 GB/s per DMA
            # engine and each dma_start engages only 2 engines, so spread
            # concurrent transfers across all five issue queues. Group-0
            # slices go first (small, unblock tile 0).
            qry_sb = inp.tile([kext, nt * 128], bf16)
            doc_sb = inp.tile([kext, totcol], bf16)
            sel_sb = inp.tile([128, 8 * nt], f16)
            gb = [gi[8] for gi in ginfo] + [totcol]  # group col offsets
            ng = len(ginfo)
            # SWDGE (gpsimd) assigns each dma_start its own DMA-engine pair
            # (~47 GB/s each, concurrent); both HWDGE queues (sync/scalar)
            # share one pair. So stream doc+qry as interleaved 2-group chunks
            # nearly all on gpsimd; sel + first doc chunk ride HWDGE.
            nc.sync.dma_start(doc_sb[:, 0 : gb[1]], docT[:, 0 : gb[1]])
            nc.gpsimd.dma_start(
                qry_sb[:, 0 : GROUP * 128], qryT[:, 0 : GROUP * 128]
            )
            for g in range(1, ng, 2):
                hi = min(g + 2, ng)
                nc.gpsimd.dma_start(
                    doc_sb[:, gb[g] : gb[hi]], docT[:, gb[g] : gb[hi]]
                )
                qa, qb = g * GROUP * 128, min(hi * GROUP, nt) * 128
                if qa < qb:
                    nc.gpsimd.dma_start(qry_sb[:, qa:qb], qryT[:, qa:qb])
            nc.scalar.dma_start(sel_sb[:], selT[:])

            negoff = accp.tile([128, 1], f32)
            nc.vector.memset(negoff[:], -OFF)
            # tiny dummy activation: pulls the Relu ACT_TABLE_LOAD into the
            # DMA head instead of stalling the first real group
            atl = accp.tile([128, 1], f16)
            nc.scalar.activation(
                atl[:], negoff[:], mybir.ActivationFunctionType.Relu,
                bias=negoff[:],
            )

            # PE warm-up during the DMA head (HAM clock ramp)
            if WARMUP_MMS:
                scratch = inp.tile([kext, 512], bf16)
                nc.vector.memset(scratch[:], 0.0)
                wps = psum.tile([128, 512], f32, tag="score")
                for _ in range(WARMUP_MMS):
                    nc.tensor.matmul(
                        wps[:], scratch[:, 0:128], scratch[:],
                        start=True, stop=True,
                    )

            accum = accp.tile([128, 16 * nt], f16)
            draw = accp.tile([128, 16 * nt], f32)
            if any(0 in gi[2] for gi in ginfo):
                # zero-width classes leave accum/draw cols unwritten
                nc.vector.memset(draw[:], 0.0)
                nc.scalar.memzero(accum[:])

            for gi, (t0, ntiles, Pks, ds, w, Ws, per_bank, nb, colofs, gcols) in (
                enumerate(ginfo)
            ):
                C = len(Pks)
                offk = [sum(w * p for p in Pks[:k]) for k in range(C)]
                nsubs = ntiles * ds
                ps = psum.tile([128, nb * 512], f32, tag="score")
                # matmuls: sub j -> bank j//per_bank, slot (j%per_bank)*Ws
                for j in range(nsubs):
                    t = t0 + j // ds
                    slot = (j // per_bank) * 512 + (j % per_bank) * Ws
                    sub = colofs + j * Ws
                    nc.tensor.matmul(
                        ps[:, slot : slot + Ws],
                        qry_sb[:, t * 128 : (t + 1) * 128],
                        doc_sb[:, sub : sub + Ws],
                        start=True,
                        stop=True,
                    )

                # chunks of subs with a regular bank pattern:
                # (bank0, nbanks, subs_per_bank, sub0)
                if ds == 1:
                    nfull = nsubs // per_bank
                    rem = nsubs % per_bank
                    chunks = []
                    if nfull:
                        chunks.append((0, nfull, per_bank, 0))
                    if rem:
                        chunks.append((nfull, 1, rem, nfull * per_bank))
                else:
                    chunks = [(0, nsubs, 1, 0)]

                def flat_view(b0, nbc, sc):
                    """[p, nb, s, Ws] strided view of the chunk's PSUM."""
                    return ps[:, b0 * 512 : (b0 + nbc) * 512].rearrange(
                        "p (nb c) -> p nb c", c=512
                    )[:, :, 0 : sc * Ws].rearrange(
                        "p nb (s c) -> p nb s c", c=Ws
                    )

                c0 = t0 * 16
                subcols = 16 // ds  # accum cols per sub
                if is_direct[gi]:
                    for b0, nbc, sc, s0 in chunks:
                        fv = flat_view(b0, nbc, sc)
                        ob = draw[
                            :, c0 + s0 * subcols : c0 + (s0 + nbc * sc) * subcols
                        ].rearrange("p (nb s c) -> p nb s c", nb=nbc, c=subcols)
                        for k in range(C):
                            if Pks[k] == 0:
                                continue
                            nc.vector.reduce_max(
                                ob[:, :, :, k * w : (k + 1) * w],
                                fv[
                                    :, :, :, offk[k] : offk[k] + w * Pks[k]
                                ].rearrange("p nb s (d t) -> p nb s d t", t=Pks[k]),
                                axis=mybir.AxisListType.X,
                            )
                    nc.vector.tensor_scalar(
                        accum[:, c0 : c0 + 16 * ntiles],
                        draw[:, c0 : c0 + 16 * ntiles],
                        OFF,
                        -OFF,
                        mybir.AluOpType.max,
                        mybir.AluOpType.add,
                    )
                else:
                    st = stp.tile([128, nsubs * Ws], f16, tag="stage")
                    for b0, nbc, sc, s0 in chunks:
                        so = st[:, s0 * Ws : (s0 + nbc * sc) * Ws].rearrange(
                            "p (nb s c) -> p nb s c", nb=nbc, c=Ws
                        )
                        nc.scalar.activation(
                            so, flat_view(b0, nbc, sc),
                            mybir.ActivationFunctionType.Relu,
                            bias=negoff[:],
                        )
                    sv = st[:].rearrange("p (a c) -> p a c", c=Ws)
                    oacc = accum[:, c0 : c0 + 16 * ntiles].rearrange(
                        "p (a c) -> p a c", c=subcols
                    )
                    for k in range(C):
                        if Pks[k] == 0:
                            continue
                        sin = sv[:, :, offk[k] : offk[k] + w * Pks[k]].rearrange(
                            "p a (d t) -> p a d t", t=Pks[k]
                        )
                        ok = oacc[:, :, k * w : (k + 1) * w]
                        if USE_POOL:
                            nc.vector.pool_max(ok, sin)
                        else:
                            nc.vector.reduce_max(
                                ok, sin, axis=mybir.AxisListType.X
                            )

            # per-q partition sums: selector matmul per supergroup of 8 tiles
            osb = accp.tile([64, 16 * nt], f16)
            for g, grp in enumerate(_supergroups(nt)):
                qts = list(grp)
                gn = len(qts)
                c0 = qts[0] * 16
                fin = fpsum.tile([8 * gn, 16 * gn], f32, tag="fin")
                nc.tensor.matmul(
                    fin[:],
                    sel_sb[:, qts[0] * 8 : (qts[-1] + 1) * 8],
                    accum[:, c0 : c0 + 16 * gn],
                    start=True,
                    stop=True,
                )
                if g % 2 == 0:
                    nc.vector.tensor_copy(osb[0 : 8 * gn, c0 : c0 + 16 * gn], fin[:])
                else:
                    nc.scalar.copy(osb[0 : 8 * gn, c0 : c0 + 16 * gn], fin[:])
                nc.sync.dma_start(
                    out[0 : 8 * gn, c0 : c0 + 16 * gn],
                    osb[0 : 8 * gn, c0 : c0 + 16 * gn],
                )
    _split_multi_waits(nc, mybir)
    return nc


def _get_nc(geom):
    _patch_ldw_opt()
    key = (geom, GROUP, DIRECT_PERIOD, DIRECT_PHASE, WARMUP_MMS)
    if key not in _CACHE:
        _CACHE[key] = _build_nc(geom)
    return _CACHE[key]


def _assemble(inputs, results, nt, perms):
    toks = np.zeros((Bq, Bd), dtype=np.float32)
    for core in range(NCORES):
        osb = np.asarray(results[core]["out"], np.float32)  # [64, 16*nt]
        part = np.zeros((Bq, BD_PER), dtype=np.float32)
        for grp in _supergroups(nt):
            for tl, t in enumerate(grp):
                part[:, perms[core][t // GROUP]] += osb[
                    8 * tl : 8 * tl + 8, t * 16 : (t + 1) * 16
                ]
        toks[:, core * BD_PER : (core + 1) * BD_PER] = part
    cls = np.asarray(inputs["qry_cls"], np.float32) @ np.asarray(
        inputs["doc_cls"], np.float32
    ).T
    scores = toks + cls
    return scores.max(axis=0).reshape(-1).astype(np.float32)


def _ensure_ntff_hook():
    """This container's antenv lacks axon_hooks; synthesize the module and
    register the ctypes-based NTFF profile hook so trace=True works."""
    import sys
    import types

    if "antenv.axon_hooks" in sys.modules:
        return
    mod = types.ModuleType("antenv.axon_hooks")
    state = {"hook": None}
    mod.set_axon_ntff_profile_hook = lambda h: state.__setitem__("hook", h)
    mod.get_axon_ntff_profile_hook = lambda: state["hook"]
    sys.modules["antenv.axon_hooks"] = mod
    try:
        import antenv

        antenv.axon_hooks = mod
    except ImportError:
        pass
    try:
        from trn_agent_boot.trn_boot import _ntff_profile_via_ctypes

        mod.set_axon_ntff_profile_hook(
            _ntff_profile_via_ctypes("/opt/axon/libaxon_pjrt.so")
        )
    except Exception:
        pass


def run(inputs, trace=False, **kwargs):
    """Run on the 8 NeuronCores; returns (output, BassKernelResults)."""
    from concourse.bass_utils import run_bass_kernel_spmd

    if trace:
        _ensure_ntff_hook()
    geom, in_maps, perms = _prepare(inputs)
    nc = _get_nc(geom)
    res = run_bass_kernel_spmd(
        nc, in_maps, core_ids=list(range(NCORES)), trace=trace, **kwargs
    )
    return _assemble(inputs, res.results, geom[2], perms), res


def kernel(**inputs) -> np.ndarray:
    out, _ = run(inputs)
    return out
